# revision 89
# baseline (speedup 1.0000x reference)
"""Single-head causal attention (B=4, T=2048, D=1024, HS=64) on 8 TRN2 cores.

Sharding: 2 cores per batch element. Query blocks (128 rows, 16/batch) are
fold-split for causal balance:
  role 0 (cores 0-3): blocks {0,1,2,3,12,13,14,15} of batch (core_id % 4)
  role 1 (cores 4-7): blocks {4..11}              of batch (core_id % 4)

Precision scheme (fp16, 10 mantissa bits):
  host: x.T in fp16 single plane (4MB/batch DMA), W pre-transposed fp16
  hi/lo pairs. k,v,q projections: 2 matmul groups (x*wh + x*wl) in fp32
  PSUM — error dominated by the single fp16 rounding of x (~2^-11 rel).
  scores: k,q re-split into fp16 hi/lo pairs on device;
    S = [qh;qh]^T.[kl;kh] + [0;ql]^T.[kl;kh]  (2 matmuls per 512-chunk)
  softmax: chunked row-max (DVE) + exp on ACT (scale=8, bias=-8*max),
  E fp16; E^T via PE transposes batched 4-per-PSUM-tile; AV inverted:
  out[q,h] accumulates lhsT=E^T-block (stationary, FWL) x rhs=v-natural
  so the result lands layout-natural and 1/Z applies per-partition.
"""

import numpy as np

N_CORES = 8
B, T, D, HS = 4, 2048, 1024, 64
P = 128
NT = T // P        # 16
ND = D // P        # 8
NCH = 4            # 512-wide t chunks
SCALE = 8.0        # sqrt(HS)
NEG = -1.0e30

ROLE_BLOCKS = [
    [0, 1, 2, 3, 12, 13, 14, 15],
    [4, 5, 6, 7, 8, 9, 10, 11],
]
ROLE_QCHUNKS = [[0, 3], [1, 2]]  # 512-chunk indices holding each role's q rows


def _block_qloc(role, j):
    if role == 0:
        return (0, 128 * j) if j < 4 else (1, 128 * (j - 12))
    return (0, 128 * (j - 4)) if j < 8 else (1, 128 * (j - 8))


_COMPILED = None


def _build():
    import concourse.bass as bass
    import concourse.tile as tile
    from concourse import bacc, mybir

    f32 = mybir.dt.float32
    f16 = mybir.dt.float16
    EXP = mybir.ActivationFunctionType.Exp
    AX = mybir.AxisListType.X

    nc = bacc.Bacc("TRN2", target_bir_lowering=False, debug=False,
                   num_devices=N_CORES)

    # x^T per batch, fp16 hi/lo planes: [2, D, T]
    xt_d = nc.dram_tensor("xt", [2, D, T], f16, kind="ExternalInput").ap()
    wkvh_d = nc.dram_tensor("wkvh", [P, ND * P], f16, kind="ExternalInput").ap()
    wkvl_d = nc.dram_tensor("wkvl", [P, ND * P], f16, kind="ExternalInput").ap()
    wqh_d = nc.dram_tensor("wqh", [P, ND * HS], f16, kind="ExternalInput").ap()
    wql_d = nc.dram_tensor("wql", [P, ND * HS], f16, kind="ExternalInput").ap()
    identb_d = nc.dram_tensor("identb", [P, P], f16, kind="ExternalInput").ap()
    mask_d = nc.dram_tensor("mask", [P, P], f32, kind="ExternalInput").ap()
    out_d = nc.dram_tensor("out", [1024, HS], f32, kind="ExternalOutput").ap()

    with tile.TileContext(nc) as tc:
        with tc.tile_pool(name="consts", bufs=1) as consts, \
             tc.tile_pool(name="big", bufs=1) as big:
            identb = consts.tile([P, P], f16)
            mask = consts.tile([P, P], f32)
            wkvh = consts.tile([P, ND, P], f16)
            wkvl = consts.tile([P, ND, P], f16)
            wqh = consts.tile([P, ND, HS], f16)
            wql = consts.tile([P, ND, HS], f16)

            # weights first: they gate the first matmul
            nc.scalar.dma_start(wkvh[:], wkvh_d.rearrange("p (a h) -> p a h", a=ND))
            nc.scalar.dma_start(wkvl[:], wkvl_d.rearrange("p (a h) -> p a h", a=ND))
            nc.scalar.dma_start(wqh[:], wqh_d.rearrange("p (a h) -> p a h", a=ND))
            nc.scalar.dma_start(wql[:], wql_d.rearrange("p (a h) -> p a h", a=ND))
            nc.scalar.dma_start(identb[:], identb_d[:])
            nc.scalar.dma_start(mask[:], mask_d[:])

            # x^T tiles, one per (plane, dt, half): [128, 1024] fp16, 2KB rows.
            # chunk ch of plane pl lives in xc[pl][dt][ch//2][:, (ch%2)*512:]
            xc = [[[big.tile([P, 1024], f16, name=f"x{pl}_{dt}_{h}",
                             tag=f"x{pl}_{dt}_{h}")
                    for h in range(2)] for dt in range(ND)] for pl in range(2)]
            # HWDGE queues only: the gpsimd SWDGE lane moves ~33GB/s and
            # starved the projection pipeline when it carried x tiles
            qi = 0
            for h in range(2):
                for dt in range(ND):
                    for pl in range(2):
                        eng = (nc.sync, nc.scalar)[qi % 2]
                        qi += 1
                        eng.dma_start(
                            xc[pl][dt][h][:],
                            xt_d[pl, dt * P:(dt + 1) * P,
                                 h * 1024:(h + 1) * 1024])

            # KHL: rows 0:64 = k_lo, rows 64:128 = k_hi
            KHL = big.tile([P, T], f16)
            vTb = big.tile([HS, T], f16)
            vn = big.tile([P, NT, HS], f16)
            qhh = big.tile([P, 2, 512], f16)   # rows 0:64=q_hi, 64:128=q_hi
            qlz = big.tile([P, 2, 512], f16)   # rows 0:64=0,    64:128=q_lo
            nc.vector.memset(qlz[0:HS, :, :], 0.0)

            # ---- k,v projections over full T (both roles) ----
            with tc.tile_pool(name="pps", bufs=2, space="PSUM") as pps, \
                 tc.tile_pool(name="kltmp", bufs=2) as klt:
                # HAM warm-up: dummy matmuls on the first-arrived weight
                # tile while x streams in, so real matmuls start at 2.4GHz
                wps = pps.tile([P, P], f32, tag="warm", bufs=1)
                for _ in range(36):
                    nc.tensor.matmul(wps[:], lhsT=wkvh[:, 0, :],
                                     rhs=wkvh[:, 0, :], start=True, stop=True)
                for ch in range(NCH):
                    cs = slice(ch * 512, (ch + 1) * 512)
                    h, c = ch // 2, ch % 2
                    ps = pps.tile([P, 512], f32, tag="proj")
                    i = 0
                    for w_t, pl in ((wkvh, 0), (wkvh, 1), (wkvl, 0)):
                        for dt in range(ND):
                            nc.tensor.matmul(
                                ps[:], lhsT=w_t[:, dt, :],
                                rhs=xc[pl][dt][h][:, c * 512:(c + 1) * 512],
                                start=(i == 0), stop=(i == 23))
                            i += 1
                    # rows 0:64 = v^T, rows 64:128 = k
                    nc.scalar.copy(vTb[:, cs], ps[0:HS, :])
                    nc.scalar.copy(KHL[HS:P, cs], ps[HS:P, :])
                    kl = klt.tile([P, 512], f16, tag="kl")
                    nc.vector.tensor_sub(kl[HS:P, :], ps[HS:P, :],
                                         KHL[HS:P, cs])
                    nc.gpsimd.dma_start(KHL[0:HS, cs], kl[HS:P, :])
                    # v^T -> v natural for this chunk's 4 key-blocks.
                    # Regular matmul against the identity instead of
                    # transpose-mode: runs at the warm 2.4GHz clock
                    # (transpose-mode never engages HAM, ~375ns each)
                    for tt in range(4 * ch, 4 * ch + 4):
                        vp = pps.tile([P, HS], f32, tag="vre")
                        nc.tensor.matmul(
                            vp[:], lhsT=vTb[:, tt * P:(tt + 1) * P],
                            rhs=identb[0:HS, 0:HS], start=True, stop=True)
                        nc.scalar.copy(vn[:, tt, :], vp[:])

            # ---- role-specific: q projections + attention ----
            with tc.tile_pool(name="spool", bufs=5, space="PSUM") as spool, \
                 tc.tile_pool(name="etp", bufs=2, space="PSUM") as etp, \
                 tc.tile_pool(name="avp", bufs=1, space="PSUM") as avp, \
                 tc.tile_pool(name="epool", bufs=3) as epool, \
                 tc.tile_pool(name="ets", bufs=4) as ets, \
                 tc.tile_pool(name="small", bufs=3) as small, \
                 tc.tile_pool(name="osb", bufs=2) as osb, \
                 tc.tile_pool(name="qtmp", bufs=2) as qtp:

                def emit_role(role):
                    for qc, ch in enumerate(ROLE_QCHUNKS[role]):
                        h, c = ch // 2, ch % 2
                        ps = spool.tile([HS, 512], f32, tag="S")
                        i = 0
                        for w_t, pl in ((wqh, 0), (wqh, 1), (wql, 0)):
                            for dt in range(ND):
                                nc.tensor.matmul(
                                    ps[:], lhsT=w_t[:, dt, :],
                                    rhs=xc[pl][dt][h][:, c * 512:(c + 1) * 512],
                                    start=(i == 0), stop=(i == 23))
                                i += 1
                        nc.scalar.copy(qhh[0:HS, qc, :], ps[:])
                        qt = qtp.tile([HS, 512], f16, tag="qt")
                        nc.vector.tensor_sub(qt[:], ps[:], qhh[0:HS, qc, :])
                        nc.gpsimd.dma_start(qhh[HS:P, qc, :], qhh[0:HS, qc, :])
                        nc.gpsimd.dma_start(qlz[HS:P, qc, :], qt[:])

                    def block_softmax(slot, j):
                        L = 128 * (j + 1)
                        qc, off = _block_qloc(role, j)
                        nch = (L + 511) // 512
                        qh_ap = qhh[:, qc, off:off + 128]
                        ql_ap = qlz[:, qc, off:off + 128]

                        sps = []
                        mc = small.tile([P, nch], f32, tag="mc", name="mc")
                        for kc in range(nch):
                            w = min(512, L - kc * 512)
                            sp = spool.tile([P, w], f32, tag="S", name="sp")
                            rhs = KHL[:, kc * 512:kc * 512 + w]
                            nc.tensor.matmul(sp[:], lhsT=qh_ap, rhs=rhs,
                                             start=True, stop=False)
                            nc.tensor.matmul(sp[:], lhsT=ql_ap, rhs=rhs,
                                             start=False, stop=True)
                            if kc == nch - 1:
                                nc.vector.tensor_add(
                                    sp[:, w - P:w], sp[:, w - P:w], mask[:])
                            nc.vector.reduce_max(
                                mc[:, kc:kc + 1], sp[:], axis=AX)
                            sps.append((sp, w))

                        nm8 = small.tile([P, 1], f32, tag="nm8", name="nm8")
                        if nch == 1:
                            nc.vector.tensor_scalar_mul(
                                nm8[:], mc[:, 0:1], -SCALE)
                        else:
                            m = small.tile([P, 1], f32, tag="m", name="m")
                            nc.vector.reduce_max(m[:], mc[:], axis=AX)
                            nc.vector.tensor_scalar_mul(nm8[:], m[:], -SCALE)

                        E = epool.tile([P, L], f16, tag="E", name="E")
                        zc = small.tile([P, nch], f32, tag="zc", name="zc")
                        for kc, (sp, w) in enumerate(sps):
                            nc.scalar.activation(
                                E[:, kc * 512:kc * 512 + w], sp[:], EXP,
                                bias=nm8[:], scale=SCALE,
                                accum_out=zc[:, kc:kc + 1])

                        rz = small.tile([P, 1], f32, tag="rz", name="rz",
                                        bufs=3)
                        if nch == 1:
                            nc.vector.reciprocal(rz[:], zc[:, 0:1])
                        else:
                            zs = small.tile([P, 1], f32, tag="zs", name="zs")
                            nc.vector.reduce_sum(zs[:], zc[:], axis=AX)
                            nc.vector.reciprocal(rz[:], zs[:])
                        return E, rz, L

                    def block_av(slot, j, E, rz, L):
                        # E^T via PE transposes, 4 per PSUM tile; AV
                        # inverted: av[q,h] += es_block.T @ vn[kt]
                        av = avp.tile([P, HS], f32, tag="av", name="av")
                        nkt = L // P
                        kt = 0
                        gi = 0
                        while kt < nkt:
                            gn = min(4, nkt - kt)
                            ep = etp.tile([P, 512], f16, name="ep")
                            for u in range(gn):
                                nc.tensor.transpose(
                                    ep[:, u * P:(u + 1) * P],
                                    E[:, (kt + u) * P:(kt + u + 1) * P],
                                    identb[:])
                            es = ets.tile([P, 512], f16, tag="ets",
                                          name="es")
                            # vector only: scalar's ACT queue is ~68% busy
                            # with chain-critical exps in the attention phase
                            nc.vector.tensor_copy(es[:, 0:gn * P],
                                                  ep[:, 0:gn * P])
                            for u in range(gn):
                                nc.tensor.matmul(
                                    av[:], lhsT=es[:, u * P:(u + 1) * P],
                                    rhs=vn[:, kt + u, :],
                                    start=(kt + u == 0),
                                    stop=(kt + u == nkt - 1),
                                    skip_group_check=True)
                            kt += gn
                            gi += 1

                        ob = osb.tile([P, HS], f32, tag="ob", name="ob")
                        nc.vector.tensor_scalar_mul(ob[:], av[:], rz[:])
                        # gpsimd queue: idle during attention; keeps the DMA
                        # trigger out of the exp-critical ACT queue
                        nc.gpsimd.dma_start(
                            out_d[slot * P:(slot + 1) * P, :], ob[:])

                    # 1-deep software pipeline: block j+1's S/max/exp phase
                    # is emitted before block j's transpose/AV phase, so the
                    # in-order PE queue streams S matmuls while the previous
                    # block's exp drains on ACT
                    pend = None
                    for slot, j in enumerate(ROLE_BLOCKS[role]):
                        cur = (slot, j) + block_softmax(slot, j)
                        if pend is not None:
                            block_av(*pend)
                        pend = cur
                    block_av(*pend)

                pid = nc.partition_id()
                with tc.If(pid < 4) as cmp:
                    emit_role(0)
                with cmp.Else():
                    emit_role(1)

    nc.compile()
    return nc


def _get_program():
    global _COMPILED
    if _COMPILED is None:
        _COMPILED = _build()
    return _COMPILED


def _install_ntff_hook():
    import sys, types
    if "antenv.axon_hooks" in sys.modules:
        return
    try:
        from trn_agent_boot.trn_boot import _ntff_profile_via_ctypes
        hook = _ntff_profile_via_ctypes("/opt/axon/libaxon_pjrt.so")
        mod = types.ModuleType("antenv.axon_hooks")
        mod.get_axon_ntff_profile_hook = lambda: hook
        mod.set_axon_ntff_profile_hook = lambda h: None
        import antenv
        sys.modules["antenv.axon_hooks"] = mod
        antenv.axon_hooks = mod
    except Exception:
        pass


def _split_pair16(a):
    hi = a.astype(np.float16)
    lo = (a - hi.astype(np.float32)).astype(np.float16)
    return hi, lo


def _host_prep(inputs):
    x = np.asarray(inputs["x"], dtype=np.float32)
    wq = np.asarray(inputs["Wq"], dtype=np.float32)
    wk = np.asarray(inputs["Wk"], dtype=np.float32)
    wv = np.asarray(inputs["Wv"], dtype=np.float32)

    xtf = np.ascontiguousarray(np.transpose(x, (0, 2, 1)))  # [B, D, T] f32
    xh, xl = _split_pair16(xtf)
    xt = np.stack([xh, xl], axis=1)                # [B, 2, D, T] fp16

    def _wprep(wt):
        # [D, M] -> [P, ND*M]: row p holds [dt, m] for d = dt*P + p
        m = wt.shape[1]
        return np.ascontiguousarray(
            wt.reshape(ND, P, m).transpose(1, 0, 2).reshape(P, ND * m))

    wkvT = np.concatenate([wv, wk], axis=0).T      # [D, 128]
    wkvh, wkvl = _split_pair16(_wprep(wkvT))
    wqT = wq.T                                     # [D, 64]
    wqh, wql = _split_pair16(_wprep(wqT))

    identb = np.eye(P, dtype=np.float16)
    r = np.arange(P)
    mask = np.where(r[None, :] <= r[:, None], 0.0, NEG).astype(np.float32)

    shared = {"wkvh": wkvh, "wkvl": wkvl, "wqh": wqh, "wql": wql,
              "identb": identb, "mask": mask}
    in_maps = []
    for c in range(N_CORES):
        m = dict(shared)
        m["xt"] = np.ascontiguousarray(xt[c % B])
        in_maps.append(m)
    return in_maps


def _run(inputs, trace=False):
    from concourse.bass_utils import run_bass_kernel_spmd

    if trace:
        _install_ntff_hook()
    nc = _get_program()
    in_maps = _host_prep(inputs)
    res = run_bass_kernel_spmd(nc, in_maps, list(range(N_CORES)), trace=trace)

    out = np.empty((B, T, HS), dtype=np.float32)
    for c in range(N_CORES):
        b, role = c % B, c // B
        oc = res.results[c]["out"]
        for slot, j in enumerate(ROLE_BLOCKS[role]):
            out[b, 128 * j:128 * (j + 1)] = oc[128 * slot:128 * (slot + 1)]
    return out, res


def kernel(**inputs):
    out, _ = _run(inputs, trace=False)
    return out
